# revision 1
# baseline (speedup 1.0000x reference)
"""Trainium2 Bass kernel: KV-memory retrieval (pool -> cosine kNN -> softmax gather).

Strategy (8 cores): shard the 65536-slot memory across cores (8192 keys/values
each) and the 256-image batch across cores (32 each) for pooling + output.

Per core, single SPMD launch:
  1. pool its x shard -> qT [512, 32]; AllGather -> qT_all [512, 256]
  2. q norms via squares + ones-matmul over partitions -> rinv [b] (per-part col)
  3. stream key blocks: row-normalize (ACT square+accum, sqrt, DVE recip),
     transpose-with-scale on PE (matmul vs diag(1/||k||)) -> kT,
     matmul1 qT.T @ kT -> sim [256, 8192], per-block top-16 candidates (max8)
  4. local top-K -> AllGather candidates -> global top-K (sorted), threshold t,
     softmax stats gmax / Z (exp with per-partition scale/bias, accum)
  5. dense w = exp(sim*rinv + bias) * (sim >= t)  (1/Z folded into bias)
  6. matmul2: w.T @ values (PE transposes of w) -> partial matched.T [512, 256]
  7. transpose -> [256, 512], ReduceScatter(add) -> own batch shard [32, 512]
  8. broadcast over 784 spatial positions, DMA out [32, 512, 784]

Selection is done on raw r = q_sum . k_norm (scale-invariant per batch row);
1/||q|| enters only through the exp scale. Mean /784 cancels everywhere.
"""

import math

import numpy as np

import concourse.bacc as bacc
import concourse.mybir as mybir
import concourse.tile as tile
from concourse.bass import ts
from concourse.bass_utils import run_bass_kernel_spmd
from concourse.masks import make_identity

F32 = mybir.dt.float32
F32R = mybir.dt.float32r
AF = mybir.ActivationFunctionType


def r32(ap):
    return ap.bitcast(F32R)

ALU = mybir.AluOpType

N_CORES = 8
NEG = -3.0e38


def build(B=256, C=512, HW=784, M=65536, K=32, n_cores=N_CORES, mb=512):
    """Build + bacc-compile the SPMD program. Returns nc."""
    BS = B // n_cores          # batches per core
    MS = M // n_cores          # memory slots per core
    CT = C // 128              # channel tiles (also contraction tiles)
    BT = B // 128 if B >= 128 else 1
    BTW = 128 if B >= 128 else B   # batch-tile width
    assert B % BTW == 0 and C % 128 == 0 and M % (n_cores * mb) == 0
    NMB = MS // mb             # key blocks per core
    KTPB = mb // 128           # 128-row key tiles per block
    R = math.ceil(K / 8)       # max8 rounds for exact top-K
    # candidates kept per 512-block: 16 suffices for K=32 at astronomically
    # high probability on continuous data; exact (R*8) for other K.
    KPB = 16 if K == 32 else min(R * 8, mb)
    RB = KPB // 8              # rounds per block
    MT = MS // 128             # value tiles
    RG = [list(range(n_cores))]
    CC_AS = "Shared" if n_cores > 4 else "Local"

    nc = bacc.Bacc("TRN2", target_bir_lowering=False, debug=False,
                   num_devices=n_cores)

    xs = nc.dram_tensor("xs", [BS, C, HW], F32, kind="ExternalInput").ap()
    keys = nc.dram_tensor("keys", [MS, C], F32, kind="ExternalInput").ap()
    vals = nc.dram_tensor("vals", [MS, C], F32, kind="ExternalInput").ap()
    out = nc.dram_tensor("out", [BS, C, HW], F32, kind="ExternalOutput").ap()

    with tile.TileContext(nc) as tc:
        with (
            tc.tile_pool(name="consts", bufs=1) as consts,
            tc.tile_pool(name="persist", bufs=1) as persist,
            tc.tile_pool(name="dram", bufs=1, space="DRAM") as dram,
        ):
            identity = consts.tile([128, 128], F32)
            make_identity(nc, identity)
            ones_col = consts.tile([128, 1], F32)
            nc.vector.memset(ones_col, 1.0)
            ones_hw = consts.tile([128, HW], F32)
            nc.vector.memset(ones_hw, 1.0)

            sim = [persist.tile([BTW, MS], F32, name=f"sim{i}")
                   for i in range(BT)]
            wexp = [persist.tile([BTW, MS], F32, name=f"wexp{i}")
                    for i in range(BT)]
            qTt = persist.tile([128, CT, B], F32, name="qTt")
            qT = [qTt[:, i] for i in range(CT)]
            qTl = [persist.tile([128, BS], F32, name=f"qTl{i}")
                   for i in range(CT)]
            cand = [persist.tile([BTW, NMB * KPB], F32, name=f"cand{i}")
                    for i in range(BT)]
            rinv = [persist.tile([BTW, 1], F32, name=f"rinv{i}")
                    for i in range(BT)]
            bias2 = [persist.tile([BTW, 1], F32, name=f"bias2{i}")
                     for i in range(BT)]
            g32 = [persist.tile([BTW, R * 8], F32, name=f"g32{i}")
                   for i in range(BT)]
            mT = [persist.tile([128, B], F32, name=f"mT{i}")
                  for i in range(CT)]
            mTmy = [persist.tile([128, BS], F32, name=f"mTmy{i}")
                    for i in range(CT)]

            # ---------------- Phase P: pool x -> qT local ----------------
            with (
                tc.tile_pool(name="poolP", bufs=3) as pP,
            ):
                hw_a = 0
                for a in range(int(math.isqrt(HW)), 1, -1):
                    if HW % a == 0:
                        hw_a = a
                        break
                CTH = CT // 2
                for b in range(BS):
                    # one DMA per batch: [C, HW] -> SBUF [128, CT, HW]
                    xt = pP.tile([128, CT, HW], F32, tag="xt")
                    nc.sync.dma_start(
                        out=xt,
                        in_=xs[b].rearrange("(ct p) hw -> p ct hw", p=128))
                    # DVE: first half of channel tiles, two-stage reduce
                    if hw_a > 1:
                        xp = pP.tile([128, CTH, HW // hw_a], F32, tag="xp")
                        nc.vector.tensor_reduce(
                            out=xp,
                            in_=xt[:, 0:CTH].rearrange(
                                "p ct (a b) -> p ct a b", a=HW // hw_a),
                            axis=mybir.AxisListType.X, op=ALU.add)
                        xq = pP.tile([128, CTH], F32, tag="xq")
                        nc.vector.tensor_reduce(
                            out=xq, in_=xp,
                            axis=mybir.AxisListType.X, op=ALU.add)
                    else:
                        xq = pP.tile([128, CTH], F32, tag="xq")
                        nc.vector.tensor_reduce(
                            out=xq, in_=xt[:, 0:CTH],
                            axis=mybir.AxisListType.X, op=ALU.add)
                    for ct in range(CTH):
                        nc.vector.tensor_copy(qTl[ct][:, b:b + 1],
                                              xq[:, ct:ct + 1])
                    # ACT: second half via square-free copy-accumulate
                    for ct in range(CTH, CT):
                        xsc = pP.tile([128, HW], F32, tag="xsc")
                        nc.scalar.activation(
                            xsc, xt[:, ct], AF.Copy,
                            accum_out=qTl[ct][:, b:b + 1])

            # ---------------- AG1: gather queries ----------------
            qag_in = dram.tile([C, BS], F32)
            qag_out = dram.tile([n_cores, C, BS], F32, addr_space=CC_AS)
            for ct in range(CT):
                nc.sync.dma_start(out=qag_in[ts(ct, 128), :], in_=qTl[ct])
            nc.gpsimd.collective_compute(
                "AllGather", ALU.bypass, replica_groups=RG,
                ins=[qag_in.opt()], outs=[qag_out.opt()])
            for r in range(n_cores):
                nc.sync.dma_start(
                    out=qTt[:, :, r * BS:(r + 1) * BS],
                    in_=qag_out[r].rearrange("(ct p) b -> p ct b", p=128))

            # ---------------- Phase Q: query norms ----------------
            with (
                tc.tile_pool(name="poolQ", bufs=2) as pQ,
                tc.tile_pool(name="psumQ", bufs=1, space="PSUM") as psQ,
            ):
                psum_ssq = psQ.tile([1, B], F32, tag="ssq")
                for ct in range(CT):
                    qsq = pQ.tile([128, B], F32, tag="qsq")
                    nc.scalar.square(qsq, qT[ct])
                    nc.tensor.matmul(psum_ssq, lhsT=ones_col, rhs=qsq,
                                     start=(ct == 0), stop=(ct == CT - 1))
                qn_row = pQ.tile([1, B], F32, tag="qn_row", bufs=1)
                nc.scalar.sqrt(qn_row, psum_ssq)
                ri_row = pQ.tile([1, B], F32, tag="ri_row", bufs=1)
                nc.vector.reciprocal(ri_row, qn_row)
                for bt in range(BT):
                    psum_rt = psQ.tile([BTW, 1], F32, tag="rt")
                    nc.tensor.matmul(
                        psum_rt, lhsT=ri_row[0:1, ts(bt, BTW)],
                        rhs=ones_col[0:1, 0:1], start=True, stop=True)
                    nc.vector.tensor_copy(rinv[bt], psum_rt)

            # ---------------- Phase K: keys -> sim + block candidates -----
            with (
                tc.tile_pool(name="poolK", bufs=2) as pK,
                tc.tile_pool(name="psumK", bufs=1, space="PSUM") as psK,
            ):
                for mbi in range(NMB):
                    kT = [pK.tile([128, mb], F32, tag=f"kT{dt}", name=f"kT{dt}")
                          for dt in range(CT)]
                    pkt = [psK.tile([128, mb], F32, tag=f"pkt{dt}", name=f"pkt{dt}")
                           for dt in range(CT)]
                    ktb = pK.tile([128, KTPB, C], F32, tag="ktb", bufs=3)
                    nc.sync.dma_start(
                        out=ktb,
                        in_=keys[mbi * mb:(mbi + 1) * mb].rearrange(
                            "(kt p) c -> p kt c", p=128))
                    for kt in range(KTPB):
                        ktile = ktb[:, kt]
                        ksq = pK.tile([128, C], F32, tag="ksq")
                        ssk = pK.tile([128, 1], F32, tag="ssk")
                        nc.scalar.activation(ksq, ktile, AF.Square,
                                             accum_out=ssk)
                        kn = pK.tile([128, 1], F32, tag="kn")
                        nc.scalar.sqrt(kn, ssk)
                        rk = pK.tile([128, 1], F32, tag="rk")
                        nc.vector.reciprocal(rk, kn)
                        dg = pK.tile([128, 128], F32, tag="dg")
                        nc.vector.tensor_scalar_mul(dg, identity, rk)
                        # transpose-with-scale: out[d, m] = k[m, d] / ||k_m||
                        for dt in range(CT):
                            nc.tensor.matmul(
                                pkt[dt][:, ts(kt, 128)],
                                lhsT=ktile[:, ts(dt, 128)], rhs=dg,
                                start=True, stop=True, skip_group_check=True)
                    for dt in range(CT):
                        if dt % 2 == 0:
                            nc.vector.tensor_copy(kT[dt], pkt[dt])
                        else:
                            nc.scalar.copy(kT[dt], pkt[dt])
                    for bt in range(BT):
                        psim = psK.tile([BTW, mb], F32, tag="psim", bufs=3)
                        for dt in range(CT):
                            nc.tensor.matmul(
                                psim, lhsT=qT[dt][:, ts(bt, BTW)],
                                rhs=kT[dt],
                                start=(dt == 0), stop=(dt == CT - 1),
                                skip_group_check=True)
                        sblk = sim[bt][:, ts(mbi, mb)]
                        if (mbi + bt) % 2 == 0:
                            nc.vector.tensor_copy(sblk, psim)
                        else:
                            nc.scalar.copy(sblk, psim)
                        # per-block top-KPB candidate values
                        cur = sblk
                        scr = pK.tile([BTW, mb], F32, tag="scr", name="scr")
                        for r in range(RB):
                            c8 = cand[bt][:, mbi * KPB + r * 8:
                                          mbi * KPB + r * 8 + 8]
                            nc.vector.max(c8, cur)
                            if r < RB - 1:
                                nc.vector.match_replace(
                                    scr, in_to_replace=c8, in_values=cur,
                                    imm_value=NEG)
                                cur = scr

            # ---------------- Phase G: global top-K + softmax stats -------
            cd_in = dram.tile([B, K], F32)
            cd_out = dram.tile([n_cores, B, K], F32, addr_space=CC_AS)
            with (
                tc.tile_pool(name="poolG", bufs=1) as pG,
            ):
                for bt in range(BT):
                    loc = pG.tile([BTW, R * 8], F32, tag="loc")
                    scr2 = pG.tile([BTW, NMB * KPB], F32, tag="scr2")
                    cur = cand[bt]
                    for r in range(R):
                        nc.vector.max(loc[:, r * 8:(r + 1) * 8], cur)
                        if r < R - 1:
                            nc.vector.match_replace(
                                scr2, in_to_replace=loc[:, r * 8:(r + 1) * 8],
                                in_values=cur, imm_value=NEG)
                            cur = scr2
                    nc.sync.dma_start(out=cd_in[ts(bt, BTW), :],
                                      in_=loc[:, 0:K])
                nc.gpsimd.collective_compute(
                    "AllGather", ALU.bypass, replica_groups=RG,
                    ins=[cd_in.opt()], outs=[cd_out.opt()])
                for bt in range(BT):
                    gc = pG.tile([BTW, n_cores * K], F32, tag="gc")
                    for r in range(n_cores):
                        nc.sync.dma_start(
                            out=gc[:, r * K:(r + 1) * K],
                            in_=cd_out[r, ts(bt, BTW), :])
                    scr3 = pG.tile([BTW, n_cores * K], F32, tag="scr3")
                    cur = gc
                    for r in range(R):
                        nc.vector.max(g32[bt][:, r * 8:(r + 1) * 8], cur)
                        if r < R - 1:
                            nc.vector.match_replace(
                                scr3,
                                in_to_replace=g32[bt][:, r * 8:(r + 1) * 8],
                                in_values=cur, imm_value=NEG)
                            cur = scr3
                    # stats: nb = -gmax*rinv ; Z = sum exp((g - gmax)*rinv)
                    nb = pG.tile([BTW, 1], F32, tag="nb")
                    nc.vector.tensor_mul(nb, g32[bt][:, 0:1], rinv[bt])
                    nc.vector.tensor_scalar_mul(nb, nb, -1.0)
                    ex = pG.tile([BTW, K], F32, tag="ex")
                    zz = pG.tile([BTW, 1], F32, tag="zz")
                    nc.scalar.activation(ex, g32[bt][:, 0:K], AF.Exp,
                                         bias=nb, scale=rinv[bt],
                                         accum_out=zz)
                    lnz = pG.tile([BTW, 1], F32, tag="lnz")
                    nc.scalar.activation(lnz, zz, AF.Ln)
                    nc.vector.tensor_sub(bias2[bt], nb, lnz)

            # ---------------- Phase W: dense weights + matmul2 ------------
            with (
                tc.tile_pool(name="poolW", bufs=2) as pW,
                tc.tile_pool(name="psumW", bufs=1, space="PSUM") as psW,
            ):
                for bt in range(BT):
                    nc.scalar.activation(wexp[bt], sim[bt], AF.Exp,
                                         bias=bias2[bt], scale=rinv[bt])
                    nc.vector.scalar_tensor_tensor(
                        out=wexp[bt], in0=sim[bt],
                        scalar=g32[bt][:, K - 1:K], in1=wexp[bt],
                        op0=ALU.is_ge, op1=ALU.mult)
                pm = [psW.tile([128, B], F32, tag=f"pm{dt}", name=f"pm{dt}")
                      for dt in range(CT)]
                VB = 4                      # value tiles per DMA
                for mt in range(MT):
                    if mt % VB == 0:
                        vtb = pW.tile([128, VB, C], F32, tag="vtb", bufs=4)
                        nc.sync.dma_start(
                            out=vtb,
                            in_=vals[mt * 128:(mt + VB) * 128].rearrange(
                                "(v p) c -> p v c", p=128))
                    vt = vtb[:, mt % VB]
                    pwt = psW.tile([128, B], F32, tag="pwt", bufs=3)
                    for bt in range(BT):
                        nc.tensor.matmul(
                            pwt[:, ts(bt, BTW)],
                            lhsT=wexp[bt][:, ts(mt, 128)],
                            rhs=identity[0:BTW, 0:BTW], is_transpose=True,
                            start=True, stop=True, skip_group_check=True)
                    wT = pW.tile([128, B], F32, tag="wT", bufs=3)
                    if mt % 2 == 0:
                        nc.vector.tensor_copy(wT, pwt)
                    else:
                        nc.scalar.copy(wT, pwt)
                    for dt in range(CT):
                        nc.tensor.matmul(
                            pm[dt], lhsT=vt[:, ts(dt, 128)], rhs=wT,
                            start=(mt == 0), stop=(mt == MT - 1),
                            skip_group_check=True)
                for dt in range(CT):
                    nc.any.tensor_copy(mT[dt], pm[dt])

            # ---------------- Phase O: reduce-scatter + broadcast out -----
            mb_dram = dram.tile([B, C], F32)
            rs_out = dram.tile([BS, C], F32)
            with (
                tc.tile_pool(name="poolO", bufs=2) as pO,
                tc.tile_pool(name="psumO", bufs=1, space="PSUM") as psO,
            ):
                for bt in range(BT):
                    pmb = psO.tile([BTW, C], F32, tag="pmb", bufs=2)
                    for dt in range(CT):
                        nc.tensor.matmul(
                            pmb[:, ts(dt, 128)], lhsT=mT[dt][:, ts(bt, BTW)],
                            rhs=identity, is_transpose=True,
                            start=True, stop=True, skip_group_check=True)
                    mrow = pO.tile([BTW, C], F32, tag="mrow")
                    nc.any.tensor_copy(mrow, pmb)
                    nc.sync.dma_start(out=mb_dram[ts(bt, BTW), :], in_=mrow)
                nc.gpsimd.collective_compute(
                    "ReduceScatter", ALU.add, replica_groups=RG,
                    ins=[mb_dram.opt()], outs=[rs_out.opt()])
                mmy = pO.tile([BS, C], F32, tag="mmy", bufs=1)
                nc.sync.dma_start(out=mmy, in_=rs_out)
                for dt in range(CT):
                    pmt = psO.tile([128, BS], F32, tag="pmt", bufs=2)
                    nc.tensor.matmul(
                        pmt, lhsT=mmy[:, ts(dt, 128)],
                        rhs=identity[0:BS, 0:BS], is_transpose=True,
                        start=True, stop=True, skip_group_check=True)
                    nc.any.tensor_copy(mTmy[dt], pmt)
                for b in range(BS):
                    ot = pO.tile([128, CT, HW], F32, tag="ot", bufs=4)
                    for dt in range(CT):
                        col = mTmy[dt][:, b:b + 1]
                        if dt < CT // 2:
                            nc.vector.tensor_scalar_mul(ot[:, dt], ones_hw,
                                                        col)
                        else:
                            nc.scalar.mul(ot[:, dt], ones_hw, col)
                    nc.sync.dma_start(
                        out=out[b].rearrange("(ct p) hw -> p ct hw", p=128),
                        in_=ot)

    nc.compile()
    return nc


_CACHE = {}
TRACE = False
LAST_RESULT = None


def _get(shape_key):
    if shape_key not in _CACHE:
        _CACHE[shape_key] = build(*shape_key)
    return _CACHE[shape_key]


def kernel(x, keys, values, topk, **_ignored):
    K = int(np.asarray(topk))
    B, C, H, W = x.shape
    M, D = keys.shape
    HW = H * W
    nc = _get((B, C, HW, M, K, N_CORES))
    BS, MS = B // N_CORES, M // N_CORES
    x3 = np.ascontiguousarray(x.reshape(B, C, HW)).astype(np.float32, copy=False)
    keys = np.ascontiguousarray(keys).astype(np.float32, copy=False)
    values = np.ascontiguousarray(values).astype(np.float32, copy=False)
    in_maps = [{
        "xs": x3[c * BS:(c + 1) * BS],
        "keys": keys[c * MS:(c + 1) * MS],
        "vals": values[c * MS:(c + 1) * MS],
    } for c in range(N_CORES)]
    global LAST_RESULT
    res = run_bass_kernel_spmd(nc, in_maps, core_ids=list(range(N_CORES)),
                               trace=TRACE)
    LAST_RESULT = res
    outs = [res.results[c]["out"] for c in range(N_CORES)]
    return np.concatenate(outs, axis=0).reshape(B, C, H, W)



# revision 13
# speedup vs baseline: 1.2524x; 1.2524x over previous
"""Trainium2 Bass kernel: KV-memory retrieval (pool -> cosine kNN -> softmax gather).

Strategy (8 cores): shard the 65536-slot memory across cores (8192 keys/values
each) and the 256-image batch across cores (32 each) for pooling + output.

Pipeline (per core, single SPMD launch) — restructured from the phase-serial
baseline to overlap the collectives and key/value streams with compute:

  1. stream x (2 batches per DMA, sync queue) -> pool -> qTl [512, 32];
     local sum-of-squares row appended; AllGather [513, 32] -> all queries
     (the AG runs while keys stream + get transposed).
  2. keys stream behind x on the sync FIFO; per 512-block: DVE square-reduce
     -> ACT sqrt -> DVE recip -> DVE row-scale, PE transposes (is_transpose,
     exact fp32) into a 12-block kT ring.  Transposes for the first 12 blocks
     sit ahead of all matmul1 in the PE queue, so they run under the AG1
     collective.
  3. matmul1 fp32 (exact — selection changes are catastrophic: one swapped
     top-32 index costs ~1.5e-2 rel err) qT.T @ kT -> sim f32 [256, 8192],
     per-block top-16 candidates (max8 rounds).
  4. local top-32 -> AllGather candidates (gpsimd queue) -> global top-32,
     threshold t, softmax stats (gmax, Z folded into exp bias).
  5. dense w = exp(sim*rinv + bias) * (sim >= t)  (all f32, exact STT mask).
  6. matmul2 in fp16 (values/weights rounded to fp16: ~5e-4 output rel err,
     no selection impact): PE transposes of w -> wT16; vals streamed f32 on
     the sync FIFO behind keys, cast to fp16 on DVE; accumulate
     vals.T @ wT -> matched.T [512, 256] in PSUM f32.
  7. transpose -> [256, 512], ReduceScatter(add) -> own batch shard [32, 512]
  8. broadcast over 784 spatial positions (DVE/ACT split), 2-batch out DMAs.

Queue routing (engine FIFOs are in-order; misplacement deadlocks or stalls):
  sync  : x, qag_in, keys 0-7, qag readback, keys 8-15, vals, mb, rs, out
  gpsimd: AG1, cd_in, AG-cand, gc readback, RS
"""

import math

import numpy as np

import concourse.bacc as bacc
import concourse.mybir as mybir
import concourse.tile as tile
from concourse.bass import ts
from concourse.bass_utils import run_bass_kernel_spmd
from concourse.masks import make_identity

F32 = mybir.dt.float32
F16 = mybir.dt.float16
AF = mybir.ActivationFunctionType
ALU = mybir.AluOpType

N_CORES = 8
NEG = -3.0e38

KT_BUFS = 11      # kT ring depth (blocks transposed ahead of matmul1)
KTB_BUFS = 2      # key-stream tiles in flight
VTB_BUFS = 4      # value-stream tiles in flight


def build(B=256, C=512, HW=784, M=65536, K=32, n_cores=N_CORES, mb=512):
    """Build + bacc-compile the SPMD program. Returns nc."""
    BS = B // n_cores          # batches per core
    MS = M // n_cores          # memory slots per core
    CT = C // 128              # channel tiles (contraction tiles)
    BT = B // 128              # batch tiles
    BTW = 128
    assert B == 256 and C == 512 and K == 32 and M % (n_cores * mb) == 0
    NMB = MS // mb             # key blocks per core
    KTPB = mb // 128           # 128-row key tiles per block
    KPB = 16                   # candidates kept per 512-block (top-16)
    MT = MS // 128             # value tiles
    RG = [list(range(n_cores))]
    CC_AS = "Shared" if n_cores > 4 else "Local"
    XPD = 2                    # batches per x DMA
    OPD = 2                    # batches per out DMA

    nc = bacc.Bacc("TRN2", target_bir_lowering=False, debug=False,
                   num_devices=n_cores)

    xs = nc.dram_tensor("xs", [BS, C, HW], F32, kind="ExternalInput").ap()
    keys = nc.dram_tensor("keys", [MS, C], F32, kind="ExternalInput").ap()
    vals = nc.dram_tensor("vals", [MS, C], F32, kind="ExternalInput").ap()
    out = nc.dram_tensor("out", [BS, C, HW], F32, kind="ExternalOutput").ap()

    with tile.TileContext(nc) as tc:
        with (
            tc.tile_pool(name="consts", bufs=1) as consts,
            tc.tile_pool(name="persist", bufs=1) as persist,
            tc.tile_pool(name="dram", bufs=1, space="DRAM") as dram,
        ):
            identity = consts.tile([128, 128], F32)
            make_identity(nc, identity)
            ones_col = consts.tile([128, 1], F32)
            nc.vector.memset(ones_col, 1.0)
            ones_hw = consts.tile([128, HW], F32)
            nc.vector.memset(ones_hw, 1.0)

            sim = [persist.tile([BTW, MS], F32, name=f"sim{i}")
                   for i in range(BT)]
            cand = [persist.tile([BTW, NMB * KPB], F32, name=f"cand{i}")
                    for i in range(BT)]
            g32 = [persist.tile([BTW, K], F32, name=f"g32{i}")
                   for i in range(BT)]
            rinv = [persist.tile([BTW, 1], F32, name=f"rinv{i}")
                    for i in range(BT)]
            bias2 = [persist.tile([BTW, 1], F32, name=f"bias2{i}")
                     for i in range(BT)]
            qTt = persist.tile([128, CT, B], F32, name="qTt")
            qTl = persist.tile([128, CT, BS], F32, name="qTl")
            ssq_l = persist.tile([1, BS], F32, name="ssq_l")
            ssq_all = persist.tile([1, B], F32, name="ssq_all")
            qn_row = persist.tile([1, B], F32, name="qn_row")
            ri_row = persist.tile([1, B], F32, name="ri_row")
            mT = persist.tile([128, CT, B], F32, name="mT")
            mTmy = [persist.tile([128, BS], F32, name=f"mTmy{i}")
                    for i in range(CT)]

            qag_in = dram.tile([C + 1, BS], F32)
            qag_out = dram.tile([n_cores, C + 1, BS], F32, addr_space=CC_AS)
            cd_in = dram.tile([B, K], F32)
            cd_out = dram.tile([n_cores, B, K], F32, addr_space=CC_AS)
            mb_dram = dram.tile([B, C], F32)
            rs_out = dram.tile([BS, C], F32)

            # ---------------- Phase P: pool x -> qTl + local ssq ----------
            hw_a = int(math.isqrt(HW))
            CTH = CT // 2
            with (
                tc.tile_pool(name="poolP", bufs=1) as pP,
                tc.tile_pool(name="psumP", bufs=1, space="PSUM") as psP,
            ):
                for xi in range(BS // XPD):
                    xt = pP.tile([128, XPD, CT, HW], F32, tag="xt", bufs=2)
                    nc.sync.dma_start(
                        out=xt,
                        in_=xs[xi * XPD:(xi + 1) * XPD].rearrange(
                            "b (ct p) hw -> p b ct hw", p=128))
                    for bs_ in range(XPD):
                        b = xi * XPD + bs_
                        # DVE: first half of channel tiles, two-stage reduce
                        xp = pP.tile([128, CTH, HW // hw_a], F32, tag="xp",
                                     bufs=2)
                        nc.vector.tensor_reduce(
                            out=xp,
                            in_=xt[:, bs_, 0:CTH].rearrange(
                                "p ct (a b) -> p ct a b", a=HW // hw_a),
                            axis=mybir.AxisListType.X, op=ALU.add)
                        xq = pP.tile([128, CTH], F32, tag="xq", bufs=2)
                        nc.vector.tensor_reduce(
                            out=xq, in_=xp,
                            axis=mybir.AxisListType.X, op=ALU.add)
                        for ct in range(CTH):
                            nc.vector.tensor_copy(qTl[:, ct, b:b + 1],
                                                  xq[:, ct:ct + 1])
                        # ACT: second half via copy-accumulate
                        for ct in range(CTH, CT):
                            xsc = pP.tile([128, HW], F32, tag="xsc", bufs=2)
                            nc.scalar.activation(
                                xsc, xt[:, bs_, ct], AF.Copy,
                                accum_out=qTl[:, ct, b:b + 1])
                # local per-batch sum of squares (for rinv after AG)
                qsq = pP.tile([128, CT, BS], F32, tag="qsq")
                nc.scalar.square(qsq, qTl)
                psq = psP.tile([1, CT * BS], F32, tag="psq")
                nc.tensor.matmul(psq, lhsT=ones_col, rhs=qsq,
                                 start=True, stop=True)
                sq_sb = pP.tile([1, CT * BS], F32, tag="sq_sb")
                nc.vector.tensor_copy(sq_sb, psq)
                nc.vector.tensor_reduce(
                    out=ssq_l,
                    in_=sq_sb.rearrange("o (ct b) -> o b ct", b=BS),
                    axis=mybir.AxisListType.X, op=ALU.add)

            # ---------------- AG1: gather queries + ssq ----------------
            for ct in range(CT):
                nc.sync.dma_start(out=qag_in[ts(ct, 128), :],
                                  in_=qTl[:, ct])
            nc.sync.dma_start(out=qag_in[C:C + 1, :], in_=ssq_l)
            nc.gpsimd.collective_compute(
                "AllGather", ALU.bypass, replica_groups=RG,
                ins=[qag_in.opt()], outs=[qag_out.opt()])

            # ---------------- Phase K: keys -> kT ring; matmul1 + topk ----
            with (
                tc.tile_pool(name="poolK", bufs=1) as pK,
                tc.tile_pool(name="psumK", bufs=1, space="PSUM") as psK,
            ):
                pkt = [psK.tile([128, mb], F32, tag=f"pkt{dt}",
                                name=f"pkt{dt}") for dt in range(CT)]
                kT_tiles = {}
                copy_flip = [0]

                def emit_mm1(j):
                    kTt = kT_tiles.pop(j)
                    for bt in range(BT):
                        psim = psK.tile([BTW, mb], F32, tag="psim", bufs=3)
                        for dt in range(CT):
                            nc.tensor.matmul(
                                psim, lhsT=qTt[:, dt, ts(bt, BTW)],
                                rhs=kTt[:, dt],
                                start=(dt == 0), stop=(dt == CT - 1),
                                skip_group_check=True)
                        sblk = sim[bt][:, ts(j, mb)]
                        if copy_flip[0] % 2 == 0:
                            nc.vector.tensor_copy(sblk, psim)
                        else:
                            nc.scalar.copy(sblk, psim)
                        copy_flip[0] += 1
                        c8a = cand[bt][:, j * KPB:j * KPB + 8]
                        c8b = cand[bt][:, j * KPB + 8:j * KPB + 16]
                        nc.vector.max(c8a, sblk)
                        scr = pK.tile([BTW, mb], F32, tag="scr", bufs=1)
                        nc.vector.match_replace(
                            scr, in_to_replace=c8a, in_values=sblk,
                            imm_value=NEG)
                        nc.vector.max(c8b, scr)

                for mbi in range(NMB):
                    if mbi >= KT_BUFS:
                        emit_mm1(mbi - KT_BUFS)
                    if mbi == NMB // 2:
                        # qag readback mid-key-stream on the sync FIFO:
                        # executes right as AG1 completes.
                        for r in range(n_cores):
                            nc.sync.dma_start(
                                out=qTt[:, :, r * BS:(r + 1) * BS],
                                in_=qag_out[r, 0:C].rearrange(
                                    "(ct p) b -> p ct b", p=128))
                        nc.sync.dma_start(
                            out=ssq_all.rearrange("o (r b) -> o r b",
                                                  r=n_cores),
                            in_=qag_out[:, C:C + 1, :].rearrange(
                                "r o b -> o r b"))
                        nc.scalar.sqrt(qn_row, ssq_all)
                        nc.vector.reciprocal(ri_row, qn_row)
                    ktb = pK.tile([128, KTPB, C], F32, tag="ktb",
                                  bufs=KTB_BUFS)
                    nc.sync.dma_start(
                        out=ktb,
                        in_=keys[mbi * mb:(mbi + 1) * mb].rearrange(
                            "(kt p) c -> p kt c", p=128))
                    kTt = pK.tile([128, CT, mb], F32, tag="kT",
                                  bufs=KT_BUFS)
                    kT_tiles[mbi] = kTt
                    for kt in range(KTPB):
                        ktile = ktb[:, kt]
                        kts = pK.tile([128, C], F32, tag="kts", bufs=2)
                        ssk = pK.tile([128, 1], F32, tag="ssk", bufs=2)
                        # kts doubles as the junk squares output here;
                        # the row-scale below overwrites it.
                        nc.scalar.activation(kts, ktile, AF.Square,
                                             accum_out=ssk)
                        kn = pK.tile([128, 1], F32, tag="kn", bufs=2)
                        nc.scalar.sqrt(kn, ssk)
                        rk = pK.tile([128, 1], F32, tag="rk", bufs=2)
                        nc.vector.reciprocal(rk, kn)
                        nc.vector.tensor_scalar_mul(kts, ktile, rk)
                        for dt in range(CT):
                            nc.tensor.matmul(
                                pkt[dt][:, ts(kt, 128)],
                                lhsT=kts[:, ts(dt, 128)], rhs=identity,
                                is_transpose=True,
                                start=True, stop=True, skip_group_check=True)
                    for dt in range(CT):
                        if dt % 2 == 0:
                            nc.vector.tensor_copy(kTt[:, dt], pkt[dt])
                        else:
                            nc.scalar.copy(kTt[:, dt], pkt[dt])
                for j in range(NMB - KT_BUFS, NMB):
                    emit_mm1(j)

            # ---------------- value stream (sync FIFO, behind keys) -------
            # Only the first VTB_BUFS value DMAs are issued ahead of the
            # candidate exchange: they fill fresh buffers and cannot stall
            # the FIFO.  The rest are emitted after the gc readback so their
            # buffer-reuse waits (on the W-phase fp16 casts) cannot block
            # cd_in/gc, which the W phase depends on.
            def emit_vtb(pV, vtbs, g):
                vtb = pV.tile([128, KTPB, C], F32, tag="vtb",
                              bufs=VTB_BUFS)
                nc.sync.dma_start(
                    out=vtb,
                    in_=vals[g * mb:(g + 1) * mb].rearrange(
                        "(kt p) c -> p kt c", p=128))
                vtbs.append(vtb)

            with tc.tile_pool(name="poolV", bufs=1) as pV:
                vtbs = []
                for g in range(VTB_BUFS):
                    emit_vtb(pV, vtbs, g)

                # ------------- Phase G: global top-K + softmax stats ------
                with (
                    tc.tile_pool(name="poolG", bufs=1) as pG,
                    tc.tile_pool(name="psumG", bufs=1, space="PSUM") as psG,
                ):
                    R = K // 8
                    for bt in range(BT):
                        psum_rt = psG.tile([BTW, 1], F32, tag="rt", bufs=2)
                        nc.tensor.matmul(
                            psum_rt, lhsT=ri_row[0:1, ts(bt, BTW)],
                            rhs=ones_col[0:1, 0:1], start=True, stop=True)
                        nc.vector.tensor_copy(rinv[bt], psum_rt)
                        loc = pG.tile([BTW, K], F32, tag="loc", bufs=2)
                        scr2 = pG.tile([BTW, NMB * KPB], F32, tag="scr2",
                                       bufs=2)
                        cur = cand[bt]
                        for r in range(R):
                            nc.vector.max(loc[:, r * 8:(r + 1) * 8], cur)
                            if r < R - 1:
                                nc.vector.match_replace(
                                    scr2,
                                    in_to_replace=loc[:, r * 8:(r + 1) * 8],
                                    in_values=cur, imm_value=NEG)
                                cur = scr2
                        nc.sync.dma_start(out=cd_in[ts(bt, BTW), :],
                                          in_=loc)
                    nc.gpsimd.collective_compute(
                        "AllGather", ALU.bypass, replica_groups=RG,
                        ins=[cd_in.opt()], outs=[cd_out.opt()])
                    for bt in range(BT):
                        gc = pG.tile([BTW, n_cores * K], F32, tag="gc",
                                     bufs=2)
                        nc.sync.dma_start(
                            out=gc.rearrange("p (r k) -> p r k", r=n_cores),
                            in_=cd_out[:, ts(bt, BTW), :].rearrange(
                                "r p k -> p r k"))
                        scr3 = pG.tile([BTW, n_cores * K], F32, tag="scr3",
                                       bufs=2)
                        cur = gc
                        for r in range(R):
                            nc.vector.max(g32[bt][:, r * 8:(r + 1) * 8], cur)
                            if r < R - 1:
                                nc.vector.match_replace(
                                    scr3,
                                    in_to_replace=g32[bt][:,
                                                          r * 8:(r + 1) * 8],
                                    in_values=cur, imm_value=NEG)
                                cur = scr3
                        # stats: nb = -gmax*rinv ; Z = sum exp((g-gmax)*rinv)
                        nb = pG.tile([BTW, 1], F32, tag="nb", bufs=2)
                        nc.vector.tensor_mul(nb, g32[bt][:, 0:1], rinv[bt])
                        nc.vector.tensor_scalar_mul(nb, nb, -1.0)
                        ex = pG.tile([BTW, K], F32, tag="ex", bufs=2)
                        zz = pG.tile([BTW, 1], F32, tag="zz", bufs=2)
                        nc.scalar.activation(ex, g32[bt][:, 0:K], AF.Exp,
                                             bias=nb, scale=rinv[bt],
                                             accum_out=zz)
                        lnz = pG.tile([BTW, 1], F32, tag="lnz", bufs=2)
                        nc.scalar.activation(lnz, zz, AF.Ln)
                        nc.vector.tensor_sub(bias2[bt], nb, lnz)

                # rest of the value stream (reuse-gated; see emit_vtb note)
                for g in range(VTB_BUFS, MT // KTPB):
                    emit_vtb(pV, vtbs, g)

                # ------------- Phase W: dense weights + matmul2 (fp16) ----
                with (
                    tc.tile_pool(name="poolW", bufs=1) as pW,
                    tc.tile_pool(name="psumW", bufs=1, space="PSUM") as psW,
                ):
                    wexp = [pW.tile([BTW, MS], F32, name=f"wexp{i}")
                            for i in range(BT)]
                    for bt in range(BT):
                        nc.scalar.activation(wexp[bt], sim[bt], AF.Exp,
                                             bias=bias2[bt], scale=rinv[bt])
                        nc.vector.scalar_tensor_tensor(
                            out=wexp[bt], in0=sim[bt],
                            scalar=g32[bt][:, K - 1:K], in1=wexp[bt],
                            op0=ALU.is_ge, op1=ALU.mult)
                    pm = [psW.tile([128, B], F32, tag=f"pm{dt}",
                                   name=f"pm{dt}") for dt in range(CT)]
                    vt16 = None
                    for mt in range(MT):
                        g, kt = mt // KTPB, mt % KTPB
                        if kt == 0:
                            vt16 = pW.tile([128, KTPB, C], F16, tag="vt16",
                                           bufs=2)
                            nc.vector.tensor_copy(vt16, vtbs[g])
                        pwt = psW.tile([128, B], F32, tag="pwt", bufs=3)
                        for bt in range(BT):
                            nc.tensor.matmul(
                                pwt[:, ts(bt, BTW)],
                                lhsT=wexp[bt][:, ts(mt, 128)],
                                rhs=identity, is_transpose=True,
                                start=True, stop=True, skip_group_check=True)
                        wT16 = pW.tile([128, B], F16, tag="wT16", bufs=3)
                        if mt % 2 == 0:
                            nc.vector.tensor_copy(wT16, pwt)
                        else:
                            nc.scalar.copy(wT16, pwt)
                        for dt in range(CT):
                            nc.tensor.matmul(
                                pm[dt], lhsT=vt16[:, kt, ts(dt, 128)],
                                rhs=wT16,
                                start=(mt == 0), stop=(mt == MT - 1),
                                skip_group_check=True)
                    for dt in range(CT):
                        nc.any.tensor_copy(mT[:, dt], pm[dt])

            # ---------------- Phase O: reduce-scatter + broadcast out -----
            with (
                tc.tile_pool(name="poolO", bufs=1) as pO,
                tc.tile_pool(name="psumO", bufs=1, space="PSUM") as psO,
            ):
                for bt in range(BT):
                    pmb = psO.tile([BTW, C], F32, tag="pmb", bufs=2)
                    for dt in range(CT):
                        nc.tensor.matmul(
                            pmb[:, ts(dt, 128)],
                            lhsT=mT[:, dt, ts(bt, BTW)],
                            rhs=identity, is_transpose=True,
                            start=True, stop=True, skip_group_check=True)
                    mrow = pO.tile([BTW, C], F32, tag="mrow", bufs=2)
                    nc.any.tensor_copy(mrow, pmb)
                    nc.sync.dma_start(out=mb_dram[ts(bt, BTW), :], in_=mrow)
                nc.gpsimd.collective_compute(
                    "ReduceScatter", ALU.add, replica_groups=RG,
                    ins=[mb_dram.opt()], outs=[rs_out.opt()])
                mmy = pO.tile([BS, C], F32, tag="mmy", bufs=1)
                nc.sync.dma_start(out=mmy, in_=rs_out)
                for dt in range(CT):
                    pmt = psO.tile([128, BS], F32, tag="pmt", bufs=2)
                    nc.tensor.matmul(
                        pmt, lhsT=mmy[:, ts(dt, 128)],
                        rhs=identity[0:BS, 0:BS], is_transpose=True,
                        start=True, stop=True, skip_group_check=True)
                    nc.any.tensor_copy(mTmy[dt], pmt)
                for oi in range(BS // OPD):
                    ot = pO.tile([128, OPD, CT, HW], F32, tag="ot", bufs=2)
                    for bs_ in range(OPD):
                        b = oi * OPD + bs_
                        for dt in range(CT):
                            col = mTmy[dt][:, b:b + 1]
                            if dt < CT // 2:
                                nc.vector.tensor_scalar_mul(
                                    ot[:, bs_, dt], ones_hw, col)
                            else:
                                nc.scalar.mul(ot[:, bs_, dt], ones_hw, col)
                    nc.sync.dma_start(
                        out=out[oi * OPD:(oi + 1) * OPD].rearrange(
                            "b (ct p) hw -> p b ct hw", p=128),
                        in_=ot)

    nc.compile()
    return nc


_CACHE = {}
TRACE = False
LAST_RESULT = None


def _get(shape_key):
    if shape_key not in _CACHE:
        _CACHE[shape_key] = build(*shape_key)
    return _CACHE[shape_key]


def kernel(x, keys, values, topk, **_ignored):
    K = int(np.asarray(topk))
    B, C, H, W = x.shape
    M, D = keys.shape
    HW = H * W
    nc = _get((B, C, HW, M, K, N_CORES))
    BS, MS = B // N_CORES, M // N_CORES
    x3 = np.ascontiguousarray(x.reshape(B, C, HW)).astype(np.float32, copy=False)
    keys = np.ascontiguousarray(keys).astype(np.float32, copy=False)
    values = np.ascontiguousarray(values).astype(np.float32, copy=False)
    in_maps = [{
        "xs": x3[c * BS:(c + 1) * BS],
        "keys": keys[c * MS:(c + 1) * MS],
        "vals": values[c * MS:(c + 1) * MS],
    } for c in range(N_CORES)]
    global LAST_RESULT
    res = run_bass_kernel_spmd(nc, in_maps, core_ids=list(range(N_CORES)),
                               trace=TRACE)
    LAST_RESULT = res
    outs = [res.results[c]["out"] for c in range(N_CORES)]
    return np.concatenate(outs, axis=0).reshape(B, C, H, W)


# revision 21
# speedup vs baseline: 1.2950x; 1.0340x over previous
"""Trainium2 Bass kernel: KV-memory retrieval (pool -> cosine kNN -> softmax gather).

Strategy (8 cores): shard the 65536-slot memory across cores (8192 keys/values
each) and the 256-image batch across cores (32 each) for pooling + output.

Pipeline (per core, single SPMD launch) — restructured from the phase-serial
baseline to overlap the collectives and key/value streams with compute:

  1. stream x (2 batches per DMA, sync queue) -> pool -> qTl [512, 32];
     local sum-of-squares row appended; AllGather [513, 32] -> all queries
     (the AG runs while keys stream + get transposed).
  2. keys stream behind x on the sync FIFO; per 512-block: DVE square-reduce
     -> ACT sqrt -> DVE recip -> DVE row-scale, PE transposes (is_transpose,
     exact fp32) into a 12-block kT ring.  Transposes for the first 12 blocks
     sit ahead of all matmul1 in the PE queue, so they run under the AG1
     collective.
  3. matmul1 fp32 (exact — selection changes are catastrophic: one swapped
     top-32 index costs ~1.5e-2 rel err) qT.T @ kT -> sim f32 [256, 8192],
     per-block top-16 candidates (max8 rounds).
  4. local top-32 -> AllGather candidates (gpsimd queue) -> global top-32,
     threshold t, softmax stats (gmax, Z folded into exp bias).
  5. dense w = exp(sim*rinv + bias) * (sim >= t)  (all f32, exact STT mask).
  6. matmul2 in fp16 (values/weights rounded to fp16: ~5e-4 output rel err,
     no selection impact): PE transposes of w -> wT16; vals streamed f32 on
     the sync FIFO behind keys, cast to fp16 on DVE; accumulate
     vals.T @ wT -> matched.T [512, 256] in PSUM f32.
  7. transpose -> [256, 512], ReduceScatter(add) -> own batch shard [32, 512]
  8. broadcast over 784 spatial positions (DVE/ACT split), 2-batch out DMAs.

Queue routing (engine FIFOs are in-order; misplacement deadlocks or stalls):
  sync  : x, qag_in, keys 0-7, qag readback, keys 8-15, vals, mb, rs, out
  gpsimd: AG1, cd_in, AG-cand, gc readback, RS
"""

import math

import numpy as np

import concourse.bacc as bacc
import concourse.mybir as mybir
import concourse.tile as tile
from concourse.bass import ts
from concourse.bass_utils import run_bass_kernel_spmd
from concourse.masks import make_identity

F32 = mybir.dt.float32
F16 = mybir.dt.float16
AF = mybir.ActivationFunctionType
ALU = mybir.AluOpType

N_CORES = 8
NEG = -3.0e38

KT_BUFS = 11      # kT ring depth (blocks transposed ahead of matmul1)
KTB_BUFS = 2      # key-stream tiles in flight
VTB_BUFS = 4      # value-stream tiles in flight


def build(B=256, C=512, HW=784, M=65536, K=32, n_cores=N_CORES, mb=512):
    """Build + bacc-compile the SPMD program. Returns nc."""
    BS = B // n_cores          # batches per core
    MS = M // n_cores          # memory slots per core
    CT = C // 128              # channel tiles (contraction tiles)
    BT = B // 128              # batch tiles
    BTW = 128
    assert B == 256 and C == 512 and K == 32 and M % (n_cores * mb) == 0
    NMB = MS // mb             # key blocks per core
    KTPB = mb // 128           # 128-row key tiles per block
    KPB = 16                   # candidates kept per 512-block (top-16)
    MT = MS // 128             # value tiles
    RG = [list(range(n_cores))]
    CC_AS = "Shared" if n_cores > 4 else "Local"
    XPD = 2                    # batches per x DMA
    OPD = 2                    # batches per out DMA

    nc = bacc.Bacc("TRN2", target_bir_lowering=False, debug=False,
                   num_devices=n_cores)

    xs = nc.dram_tensor("xs", [BS, C, HW], F32, kind="ExternalInput").ap()
    keys = nc.dram_tensor("keys", [MS, C], F32, kind="ExternalInput").ap()
    vals = nc.dram_tensor("vals", [MS, C], F32, kind="ExternalInput").ap()
    out = nc.dram_tensor("out", [BS, C, HW], F32, kind="ExternalOutput").ap()

    with tile.TileContext(nc) as tc:
        with (
            tc.tile_pool(name="consts", bufs=1) as consts,
            tc.tile_pool(name="persist", bufs=1) as persist,
            tc.tile_pool(name="dram", bufs=1, space="DRAM") as dram,
        ):
            identity = consts.tile([128, 128], F32)
            make_identity(nc, identity)
            ones_col = consts.tile([128, 1], F32)
            nc.vector.memset(ones_col, 1.0)
            ones_hw = consts.tile([128, HW], F32)
            nc.vector.memset(ones_hw, 1.0)

            sim = [persist.tile([BTW, MS], F32, name=f"sim{i}")
                   for i in range(BT)]
            cand = [persist.tile([BTW, NMB * KPB], F32, name=f"cand{i}")
                    for i in range(BT)]
            g32 = [persist.tile([BTW, K], F32, name=f"g32{i}")
                   for i in range(BT)]
            rinv = [persist.tile([BTW, 1], F32, name=f"rinv{i}")
                    for i in range(BT)]
            bias2 = [persist.tile([BTW, 1], F32, name=f"bias2{i}")
                     for i in range(BT)]
            nb_l = [persist.tile([BTW, 1], F32, name=f"nb_l{i}")
                    for i in range(BT)]
            rowfix = [persist.tile([BTW, 1], F32, name=f"rowfix{i}")
                      for i in range(BT)]
            qTt = persist.tile([128, CT, B], F32, name="qTt")
            qTl = persist.tile([128, CT, BS], F32, name="qTl")
            qn_row = persist.tile([1, B], F32, name="qn_row")
            ri_row = persist.tile([1, B], F32, name="ri_row")
            mT = persist.tile([128, CT, B], F32, name="mT")
            mTmy = [persist.tile([128, BS], F32, name=f"mTmy{i}")
                    for i in range(CT)]

            BS2 = BS // 2
            qag_in = [dram.tile([C, BS2], F32, name=f"qag_in{h}")
                      for h in range(2)]
            qag_out = [dram.tile([n_cores, C, BS2], F32, addr_space=CC_AS,
                                 name=f"qag_out{h}")
                       for h in range(2)]
            cd_in = dram.tile([B, K], F32)
            cd_out = dram.tile([n_cores, B, K], F32, addr_space=CC_AS)
            mb_dram = dram.tile([B, C], F32)
            rs_out = dram.tile([BS, C], F32)

            def emit_ag1(h):
                # AllGather queries for batch half h; dispatched early so the
                # ~40us collective dispatch latency hides under pooling.
                for ct in range(CT):
                    nc.sync.dma_start(
                        out=qag_in[h][ts(ct, 128), :],
                        in_=qTl[:, ct, h * BS2:(h + 1) * BS2])
                nc.gpsimd.collective_compute(
                    "AllGather", ALU.bypass, replica_groups=RG,
                    ins=[qag_in[h].opt()], outs=[qag_out[h].opt()])

            # ---------------- Phase P: pool x -> qTl + local ssq ----------
            hw_a = int(math.isqrt(HW))
            CTH = CT // 2
            with (
                tc.tile_pool(name="poolP", bufs=1) as pP,
            ):
                for xi in range(BS // XPD):
                    if xi * XPD == BS2:
                        emit_ag1(0)
                    xt = pP.tile([128, XPD, CT, HW], F32, tag="xt", bufs=2)
                    nc.sync.dma_start(
                        out=xt,
                        in_=xs[xi * XPD:(xi + 1) * XPD].rearrange(
                            "b (ct p) hw -> p b ct hw", p=128))
                    for bs_ in range(XPD):
                        b = xi * XPD + bs_
                        # DVE: first half of channel tiles, two-stage reduce
                        xp = pP.tile([128, CTH, HW // hw_a], F32, tag="xp",
                                     bufs=2)
                        nc.vector.tensor_reduce(
                            out=xp,
                            in_=xt[:, bs_, 0:CTH].rearrange(
                                "p ct (a b) -> p ct a b", a=HW // hw_a),
                            axis=mybir.AxisListType.X, op=ALU.add)
                        xq = pP.tile([128, CTH], F32, tag="xq", bufs=2)
                        nc.vector.tensor_reduce(
                            out=xq, in_=xp,
                            axis=mybir.AxisListType.X, op=ALU.add)
                        for ct in range(CTH):
                            nc.vector.tensor_copy(qTl[:, ct, b:b + 1],
                                                  xq[:, ct:ct + 1])
                        # ACT: second half via copy-accumulate
                        for ct in range(CTH, CT):
                            xsc = pP.tile([128, HW], F32, tag="xsc", bufs=2)
                            nc.scalar.activation(
                                xsc, xt[:, bs_, ct], AF.Copy,
                                accum_out=qTl[:, ct, b:b + 1])
            # ---------------- AG1b: second batch half ----------------
            emit_ag1(1)

            # ---------------- Phase K: keys -> kT ring; matmul1 + topk ----
            with (
                tc.tile_pool(name="poolK", bufs=1) as pK,
                tc.tile_pool(name="psumK", bufs=1, space="PSUM") as psK,
            ):
                pkt = [psK.tile([128, mb], F32, tag=f"pkt{dt}",
                                name=f"pkt{dt}") for dt in range(CT)]
                kT_tiles = {}
                copy_flip = [0]

                def emit_mm1(j):
                    kTt = kT_tiles.pop(j)
                    for bt in range(BT):
                        psim = psK.tile([BTW, mb], F32, tag="psim", bufs=3)
                        for dt in range(CT):
                            nc.tensor.matmul(
                                psim, lhsT=qTt[:, dt, ts(bt, BTW)],
                                rhs=kTt[:, dt],
                                start=(dt == 0), stop=(dt == CT - 1),
                                skip_group_check=True)
                        sblk = sim[bt][:, ts(j, mb)]
                        if copy_flip[0] % 2 == 0:
                            nc.vector.tensor_copy(sblk, psim)
                        else:
                            nc.scalar.copy(sblk, psim)
                        copy_flip[0] += 1
                        c8a = cand[bt][:, j * KPB:j * KPB + 8]
                        c8b = cand[bt][:, j * KPB + 8:j * KPB + 16]
                        nc.vector.max(c8a, sblk)
                        scr = pK.tile([BTW, mb], F32, tag="scr", bufs=1)
                        nc.vector.match_replace(
                            scr, in_to_replace=c8a, in_values=sblk,
                            imm_value=NEG)
                        nc.vector.max(c8b, scr)

                for mbi in range(NMB):
                    if mbi >= KT_BUFS:
                        emit_mm1(mbi - KT_BUFS)
                    if mbi == NMB // 2 + 2:
                        # qag readback mid-key-stream on the sync FIFO:
                        # executes right as AG1b completes.
                        for h in range(2):
                            for r in range(n_cores):
                                nc.sync.dma_start(
                                    out=qTt[:, :,
                                            r * BS + h * BS2:
                                            r * BS + (h + 1) * BS2],
                                    in_=qag_out[h][r].rearrange(
                                        "(ct p) b -> p ct b", p=128))
                    ktb = pK.tile([128, KTPB, C], F32, tag="ktb",
                                  bufs=KTB_BUFS)
                    nc.sync.dma_start(
                        out=ktb,
                        in_=keys[mbi * mb:(mbi + 1) * mb].rearrange(
                            "(kt p) c -> p kt c", p=128))
                    kTt = pK.tile([128, CT, mb], F32, tag="kT",
                                  bufs=KT_BUFS)
                    kT_tiles[mbi] = kTt
                    for kt in range(KTPB):
                        ktile = ktb[:, kt]
                        kts = pK.tile([128, C], F32, tag="kts", bufs=2)
                        ssk = pK.tile([128, 1], F32, tag="ssk", bufs=2)
                        # kts doubles as the junk squares output here;
                        # the row-scale below overwrites it.
                        nc.scalar.activation(kts, ktile, AF.Square,
                                             accum_out=ssk)
                        kn = pK.tile([128, 1], F32, tag="kn", bufs=2)
                        nc.scalar.sqrt(kn, ssk)
                        rk = pK.tile([128, 1], F32, tag="rk", bufs=2)
                        nc.vector.reciprocal(rk, kn)
                        nc.vector.tensor_scalar_mul(kts, ktile, rk)
                        for dt in range(CT):
                            nc.tensor.matmul(
                                pkt[dt][:, ts(kt, 128)],
                                lhsT=kts[:, ts(dt, 128)], rhs=identity,
                                is_transpose=True,
                                start=True, stop=True, skip_group_check=True)
                    for dt in range(CT):
                        if dt % 2 == 0:
                            nc.vector.tensor_copy(kTt[:, dt], pkt[dt])
                        else:
                            nc.scalar.copy(kTt[:, dt], pkt[dt])
                for j in range(NMB - KT_BUFS, NMB):
                    emit_mm1(j)

            # ---------------- value stream (sync FIFO, behind keys) -------
            # Only the first VTB_BUFS value DMAs are issued ahead of the
            # candidate exchange: they fill fresh buffers and cannot stall
            # the FIFO.  The rest are emitted after the gc readback so their
            # buffer-reuse waits (on the W-phase fp16 casts) cannot block
            # cd_in/gc, which the W phase depends on.
            def emit_vtb(pV, vtbs, g):
                vtb = pV.tile([128, KTPB, C], F32, tag="vtb",
                              bufs=VTB_BUFS)
                nc.sync.dma_start(
                    out=vtb,
                    in_=vals[g * mb:(g + 1) * mb].rearrange(
                        "(kt p) c -> p kt c", p=128))
                vtbs.append(vtb)

            with tc.tile_pool(name="poolV", bufs=1) as pV:
                vtbs = []
                for g in range(VTB_BUFS):
                    emit_vtb(pV, vtbs, g)
                wexp = [pV.tile([BTW, MS], F32, name=f"wexp{i}")
                        for i in range(BT)]

                # ------------- Phase G: global top-K + softmax stats ------
                with (
                    tc.tile_pool(name="poolG", bufs=1) as pG,
                    tc.tile_pool(name="psumG", bufs=1, space="PSUM") as psG,
                ):
                    R = K // 8
                    # Q: rinv from the gathered queries
                    qsq = pG.tile([128, CT, B], F32, tag="qsq")
                    nc.scalar.square(qsq, qTt)
                    pss = psG.tile([1, B], F32, tag="pss")
                    for ct in range(CT):
                        nc.tensor.matmul(pss, lhsT=ones_col, rhs=qsq[:, ct],
                                         start=(ct == 0), stop=(ct == CT - 1))
                    nc.scalar.sqrt(qn_row, pss)
                    nc.vector.reciprocal(ri_row, qn_row)
                    for bt in range(BT):
                        psum_rt = psG.tile([BTW, 1], F32, tag="rt", bufs=2)
                        nc.tensor.matmul(
                            psum_rt, lhsT=ri_row[0:1, ts(bt, BTW)],
                            rhs=ones_col[0:1, 0:1], start=True, stop=True)
                        nc.vector.tensor_copy(rinv[bt], psum_rt)
                        loc = pG.tile([BTW, K], F32, tag="loc", bufs=2)
                        scr2 = pG.tile([BTW, NMB * KPB], F32, tag="scr2",
                                       bufs=2)
                        cur = cand[bt]
                        for r in range(R):
                            nc.vector.max(loc[:, r * 8:(r + 1) * 8], cur)
                            if r < R - 1:
                                nc.vector.match_replace(
                                    scr2,
                                    in_to_replace=loc[:, r * 8:(r + 1) * 8],
                                    in_values=cur, imm_value=NEG)
                                cur = scr2
                        nc.gpsimd.dma_start(out=cd_in[ts(bt, BTW), :],
                                            in_=loc)
                        # local softmax bias: nb_l = -lmax * rinv
                        nc.vector.tensor_mul(nb_l[bt], loc[:, 0:1], rinv[bt])
                        nc.vector.tensor_scalar_mul(nb_l[bt], nb_l[bt], -1.0)
                    # exp with LOCAL stats — runs under the AG-cand latency;
                    # the global correction folds into rowfix (phase O).
                    for bt in range(BT):
                        nc.scalar.activation(wexp[bt], sim[bt], AF.Exp,
                                             bias=nb_l[bt], scale=rinv[bt])
                    nc.gpsimd.collective_compute(
                        "AllGather", ALU.bypass, replica_groups=RG,
                        ins=[cd_in.opt()], outs=[cd_out.opt()])
                    for bt in range(BT):
                        gc = pG.tile([BTW, n_cores * K], F32, tag="gc",
                                     bufs=2)
                        nc.gpsimd.dma_start(
                            out=gc.rearrange("p (r k) -> p r k", r=n_cores),
                            in_=cd_out[:, ts(bt, BTW), :].rearrange(
                                "r p k -> p r k"))
                        scr3 = pG.tile([BTW, n_cores * K], F32, tag="scr3",
                                       bufs=2)
                        cur = gc
                        for r in range(R):
                            nc.vector.max(g32[bt][:, r * 8:(r + 1) * 8], cur)
                            if r < R - 1:
                                nc.vector.match_replace(
                                    scr3,
                                    in_to_replace=g32[bt][:,
                                                          r * 8:(r + 1) * 8],
                                    in_values=cur, imm_value=NEG)
                                cur = scr3
                        # stats: nbg = -gmax*rinv ; Z = sum exp((g-gmax)*rinv)
                        # rowfix = exp(nbg - nb_l - lnZ) applied in phase O
                        nbg = pG.tile([BTW, 1], F32, tag="nbg", bufs=2)
                        nc.vector.tensor_mul(nbg, g32[bt][:, 0:1], rinv[bt])
                        nc.vector.tensor_scalar_mul(nbg, nbg, -1.0)
                        ex = pG.tile([BTW, K], F32, tag="ex", bufs=2)
                        zz = pG.tile([BTW, 1], F32, tag="zz", bufs=2)
                        nc.scalar.activation(ex, g32[bt][:, 0:K], AF.Exp,
                                             bias=nbg, scale=rinv[bt],
                                             accum_out=zz)
                        lnz = pG.tile([BTW, 1], F32, tag="lnz", bufs=2)
                        nc.scalar.activation(lnz, zz, AF.Ln)
                        nc.vector.tensor_sub(bias2[bt], nbg, nb_l[bt])
                        nc.vector.tensor_sub(bias2[bt], bias2[bt], lnz)
                        nc.scalar.activation(rowfix[bt], bias2[bt], AF.Exp)

                # rest of the value stream (reuse-gated; see emit_vtb note)
                for g in range(VTB_BUFS, MT // KTPB):
                    emit_vtb(pV, vtbs, g)

                # ------------- Phase W: dense weights + matmul2 (fp16) ----
                with (
                    tc.tile_pool(name="poolW", bufs=1) as pW,
                    tc.tile_pool(name="psumW", bufs=1, space="PSUM") as psW,
                ):
                    for bt in range(BT):
                        nc.vector.scalar_tensor_tensor(
                            out=wexp[bt], in0=sim[bt],
                            scalar=g32[bt][:, K - 1:K], in1=wexp[bt],
                            op0=ALU.is_ge, op1=ALU.mult)
                    pm = [psW.tile([128, B], F32, tag=f"pm{dt}",
                                   name=f"pm{dt}") for dt in range(CT)]
                    vt16 = None
                    for mt in range(MT):
                        g, kt = mt // KTPB, mt % KTPB
                        if kt == 0:
                            vt16 = pW.tile([128, KTPB, C], F16, tag="vt16",
                                           bufs=2)
                            nc.vector.tensor_copy(vt16, vtbs[g])
                        pwt = psW.tile([128, B], F32, tag="pwt", bufs=3)
                        for bt in range(BT):
                            nc.tensor.matmul(
                                pwt[:, ts(bt, BTW)],
                                lhsT=wexp[bt][:, ts(mt, 128)],
                                rhs=identity, is_transpose=True,
                                start=True, stop=True, skip_group_check=True)
                        wT16 = pW.tile([128, B], F16, tag="wT16", bufs=3)
                        if mt % 2 == 0:
                            nc.vector.tensor_copy(wT16, pwt)
                        else:
                            nc.scalar.copy(wT16, pwt)
                        for dt in range(CT):
                            nc.tensor.matmul(
                                pm[dt], lhsT=vt16[:, kt, ts(dt, 128)],
                                rhs=wT16,
                                start=(mt == 0), stop=(mt == MT - 1),
                                skip_group_check=True)
                    for dt in range(CT):
                        nc.any.tensor_copy(mT[:, dt], pm[dt])

            # ---------------- Phase O: reduce-scatter + broadcast out -----
            with (
                tc.tile_pool(name="poolO", bufs=1) as pO,
                tc.tile_pool(name="psumO", bufs=1, space="PSUM") as psO,
            ):
                for bt in range(BT):
                    pmb = psO.tile([BTW, C], F32, tag="pmb", bufs=2)
                    for dt in range(CT):
                        nc.tensor.matmul(
                            pmb[:, ts(dt, 128)],
                            lhsT=mT[:, dt, ts(bt, BTW)],
                            rhs=identity, is_transpose=True,
                            start=True, stop=True, skip_group_check=True)
                    mrow = pO.tile([BTW, C], F32, tag="mrow", bufs=2)
                    # fold the local->global softmax correction in here
                    nc.scalar.mul(mrow, pmb, rowfix[bt])
                    nc.sync.dma_start(out=mb_dram[ts(bt, BTW), :], in_=mrow)
                nc.gpsimd.collective_compute(
                    "ReduceScatter", ALU.add, replica_groups=RG,
                    ins=[mb_dram.opt()], outs=[rs_out.opt()])
                mmy = pO.tile([BS, C], F32, tag="mmy", bufs=1)
                nc.sync.dma_start(out=mmy, in_=rs_out)
                for dt in range(CT):
                    pmt = psO.tile([128, BS], F32, tag="pmt", bufs=2)
                    nc.tensor.matmul(
                        pmt, lhsT=mmy[:, ts(dt, 128)],
                        rhs=identity[0:BS, 0:BS], is_transpose=True,
                        start=True, stop=True, skip_group_check=True)
                    nc.any.tensor_copy(mTmy[dt], pmt)
                for oi in range(BS // OPD):
                    ot = pO.tile([128, OPD, CT, HW], F32, tag="ot", bufs=2)
                    for bs_ in range(OPD):
                        b = oi * OPD + bs_
                        for dt in range(CT):
                            col = mTmy[dt][:, b:b + 1]
                            if dt < CT // 2:
                                nc.vector.tensor_scalar_mul(
                                    ot[:, bs_, dt], ones_hw, col)
                            else:
                                nc.scalar.mul(ot[:, bs_, dt], ones_hw, col)
                    nc.sync.dma_start(
                        out=out[oi * OPD:(oi + 1) * OPD].rearrange(
                            "b (ct p) hw -> p b ct hw", p=128),
                        in_=ot)

    nc.compile()
    return nc


_CACHE = {}
TRACE = False
LAST_RESULT = None


def _get(shape_key):
    if shape_key not in _CACHE:
        _CACHE[shape_key] = build(*shape_key)
    return _CACHE[shape_key]


def kernel(x, keys, values, topk, **_ignored):
    K = int(np.asarray(topk))
    B, C, H, W = x.shape
    M, D = keys.shape
    HW = H * W
    nc = _get((B, C, HW, M, K, N_CORES))
    BS, MS = B // N_CORES, M // N_CORES
    x3 = np.ascontiguousarray(x.reshape(B, C, HW)).astype(np.float32, copy=False)
    keys = np.ascontiguousarray(keys).astype(np.float32, copy=False)
    values = np.ascontiguousarray(values).astype(np.float32, copy=False)
    in_maps = [{
        "xs": x3[c * BS:(c + 1) * BS],
        "keys": keys[c * MS:(c + 1) * MS],
        "vals": values[c * MS:(c + 1) * MS],
    } for c in range(N_CORES)]
    global LAST_RESULT
    res = run_bass_kernel_spmd(nc, in_maps, core_ids=list(range(N_CORES)),
                               trace=TRACE)
    LAST_RESULT = res
    outs = [res.results[c]["out"] for c in range(N_CORES)]
    return np.concatenate(outs, axis=0).reshape(B, C, H, W)


# revision 25
# speedup vs baseline: 1.3042x; 1.0071x over previous
"""Trainium2 Bass kernel: KV-memory retrieval (pool -> cosine kNN -> softmax gather).

Strategy (8 cores): shard the 65536-slot memory across cores (8192 keys/values
each) and the 256-image batch across cores (32 each) for pooling + output.

Pipeline (per core, single SPMD launch) — restructured from the phase-serial
baseline to overlap the collectives and key/value streams with compute:

  1. stream x (2 batches per DMA, sync queue) -> pool -> qTl [512, 32];
     local sum-of-squares row appended; AllGather [513, 32] -> all queries
     (the AG runs while keys stream + get transposed).
  2. keys stream behind x on the sync FIFO; per 512-block: DVE square-reduce
     -> ACT sqrt -> DVE recip -> DVE row-scale, PE transposes (is_transpose,
     exact fp32) into a 12-block kT ring.  Transposes for the first 12 blocks
     sit ahead of all matmul1 in the PE queue, so they run under the AG1
     collective.
  3. matmul1 fp32 (exact — selection changes are catastrophic: one swapped
     top-32 index costs ~1.5e-2 rel err) qT.T @ kT -> sim f32 [256, 8192],
     per-block top-16 candidates (max8 rounds).
  4. local top-32 -> AllGather candidates (gpsimd queue) -> global top-32,
     threshold t, softmax stats (gmax, Z folded into exp bias).
  5. dense w = exp(sim*rinv + bias) * (sim >= t)  (all f32, exact STT mask).
  6. matmul2 in fp16 (values/weights rounded to fp16: ~5e-4 output rel err,
     no selection impact): PE transposes of w -> wT16; vals streamed f32 on
     the sync FIFO behind keys, cast to fp16 on DVE; accumulate
     vals.T @ wT -> matched.T [512, 256] in PSUM f32.
  7. transpose -> [256, 512], ReduceScatter(add) -> own batch shard [32, 512]
  8. broadcast over 784 spatial positions (DVE/ACT split), 2-batch out DMAs.

Queue routing (engine FIFOs are in-order; misplacement deadlocks or stalls):
  sync  : x, qag_in, keys 0-7, qag readback, keys 8-15, vals, mb, rs, out
  gpsimd: AG1, cd_in, AG-cand, gc readback, RS
"""

import math

import numpy as np

import concourse.bacc as bacc
import concourse.mybir as mybir
import concourse.tile as tile
from concourse.bass import ts
from concourse.bass_utils import run_bass_kernel_spmd
from concourse.masks import make_identity

F32 = mybir.dt.float32
F16 = mybir.dt.float16
AF = mybir.ActivationFunctionType
ALU = mybir.AluOpType

N_CORES = 8
NEG = -3.0e38

KT_BUFS = 11      # kT ring depth (blocks transposed ahead of matmul1)
KTB_BUFS = 2      # key-stream tiles in flight
VTB_BUFS = 4      # value-stream tiles in flight


def build(B=256, C=512, HW=784, M=65536, K=32, n_cores=N_CORES, mb=512):
    """Build + bacc-compile the SPMD program. Returns nc."""
    BS = B // n_cores          # batches per core
    MS = M // n_cores          # memory slots per core
    CT = C // 128              # channel tiles (contraction tiles)
    BT = B // 128              # batch tiles
    BTW = 128
    assert B == 256 and C == 512 and K == 32 and M % (n_cores * mb) == 0
    NMB = MS // mb             # key blocks per core
    KTPB = mb // 128           # 128-row key tiles per block
    KPB = 16                   # candidates kept per 512-block (top-16)
    MT = MS // 128             # value tiles
    RG = [list(range(n_cores))]
    CC_AS = "Shared" if n_cores > 4 else "Local"
    XPD = 2                    # batches per x DMA
    OPD = 2                    # batches per out DMA

    nc = bacc.Bacc("TRN2", target_bir_lowering=False, debug=False,
                   num_devices=n_cores)

    xs = nc.dram_tensor("xs", [BS, C, HW], F32, kind="ExternalInput").ap()
    keys = nc.dram_tensor("keys", [MS, C], F32, kind="ExternalInput").ap()
    vals = nc.dram_tensor("vals", [MS, C], F32, kind="ExternalInput").ap()
    out = nc.dram_tensor("out", [BS, C, HW], F32, kind="ExternalOutput").ap()

    with tile.TileContext(nc) as tc:
        with (
            tc.tile_pool(name="consts", bufs=1) as consts,
            tc.tile_pool(name="persist", bufs=1) as persist,
            tc.tile_pool(name="dram", bufs=1, space="DRAM") as dram,
        ):
            identity = consts.tile([128, 128], F32)
            make_identity(nc, identity)
            ones_col = consts.tile([128, 1], F32)
            nc.vector.memset(ones_col, 1.0)
            ones_hw = consts.tile([128, HW], F32)
            nc.vector.memset(ones_hw, 1.0)

            sim = [persist.tile([BTW, MS], F32, name=f"sim{i}")
                   for i in range(BT)]
            cand = [persist.tile([BTW, NMB * KPB], F32, name=f"cand{i}")
                    for i in range(BT)]
            g32 = [persist.tile([BTW, K], F32, name=f"g32{i}")
                   for i in range(BT)]
            rinv = [persist.tile([BTW, 1], F32, name=f"rinv{i}")
                    for i in range(BT)]
            bias2 = [persist.tile([BTW, 1], F32, name=f"bias2{i}")
                     for i in range(BT)]
            nb_l = [persist.tile([BTW, 1], F32, name=f"nb_l{i}")
                    for i in range(BT)]
            rowfix = [persist.tile([BTW, 1], F32, name=f"rowfix{i}")
                      for i in range(BT)]
            qTt = persist.tile([128, CT, B], F32, name="qTt")
            qTl = persist.tile([128, CT, BS], F32, name="qTl")
            qn_row = persist.tile([1, B], F32, name="qn_row")
            ri_row = persist.tile([1, B], F32, name="ri_row")
            mT = persist.tile([128, CT, B], F32, name="mT")
            mTmy = [persist.tile([128, BS], F32, name=f"mTmy{i}")
                    for i in range(CT)]

            BS2 = BS // 2
            qag_in = [dram.tile([C, BS2], F32, name=f"qag_in{h}")
                      for h in range(2)]
            qag_out = [dram.tile([n_cores, C, BS2], F32, addr_space=CC_AS,
                                 name=f"qag_out{h}")
                       for h in range(2)]
            cd_in = dram.tile([B, K], F32)
            cd_out = dram.tile([n_cores, B, K], F32, addr_space=CC_AS)
            mb_dram = dram.tile([B, C], F32)
            rs_out = dram.tile([BS, C], F32)

            def emit_ag1(h):
                # AllGather queries for batch half h; dispatched early so the
                # ~40us collective dispatch latency hides under pooling.
                for ct in range(CT):
                    nc.sync.dma_start(
                        out=qag_in[h][ts(ct, 128), :],
                        in_=qTl[:, ct, h * BS2:(h + 1) * BS2])
                nc.gpsimd.collective_compute(
                    "AllGather", ALU.bypass, replica_groups=RG,
                    ins=[qag_in[h].opt()], outs=[qag_out[h].opt()])

            # ---------------- Phase P: pool x -> qTl + local ssq ----------
            hw_a = int(math.isqrt(HW))
            CTH = CT // 2
            with (
                tc.tile_pool(name="poolP", bufs=1) as pP,
            ):
                for xi in range(BS // XPD):
                    if xi * XPD == BS2:
                        emit_ag1(0)
                    xt = pP.tile([128, XPD, CT, HW], F32, tag="xt", bufs=2)
                    nc.sync.dma_start(
                        out=xt,
                        in_=xs[xi * XPD:(xi + 1) * XPD].rearrange(
                            "b (ct p) hw -> p b ct hw", p=128))
                    for bs_ in range(XPD):
                        b = xi * XPD + bs_
                        # DVE: first half of channel tiles, two-stage reduce
                        xp = pP.tile([128, CTH, HW // hw_a], F32, tag="xp",
                                     bufs=2)
                        nc.vector.tensor_reduce(
                            out=xp,
                            in_=xt[:, bs_, 0:CTH].rearrange(
                                "p ct (a b) -> p ct a b", a=HW // hw_a),
                            axis=mybir.AxisListType.X, op=ALU.add)
                        xq = pP.tile([128, CTH], F32, tag="xq", bufs=2)
                        nc.vector.tensor_reduce(
                            out=xq, in_=xp,
                            axis=mybir.AxisListType.X, op=ALU.add)
                        for ct in range(CTH):
                            nc.vector.tensor_copy(qTl[:, ct, b:b + 1],
                                                  xq[:, ct:ct + 1])
                        # ACT: second half via copy-accumulate
                        for ct in range(CTH, CT):
                            xsc = pP.tile([128, HW], F32, tag="xsc", bufs=2)
                            nc.scalar.activation(
                                xsc, xt[:, bs_, ct], AF.Copy,
                                accum_out=qTl[:, ct, b:b + 1])
            # ---------------- AG1b: second batch half ----------------
            emit_ag1(1)

            # ---------------- Phase K: keys -> kT ring; matmul1 + topk ----
            with (
                tc.tile_pool(name="poolK", bufs=1) as pK,
                tc.tile_pool(name="psumK", bufs=1, space="PSUM") as psK,
            ):
                pkt = [psK.tile([128, mb], F32, tag=f"pkt{dt}",
                                name=f"pkt{dt}") for dt in range(CT)]
                kT_tiles = {}
                copy_flip = [0]

                def emit_mm1(j):
                    kTt = kT_tiles.pop(j)
                    for bt in range(BT):
                        psim = psK.tile([BTW, mb], F32, tag="psim", bufs=3)
                        for dt in range(CT):
                            nc.tensor.matmul(
                                psim, lhsT=qTt[:, dt, ts(bt, BTW)],
                                rhs=kTt[:, dt],
                                start=(dt == 0), stop=(dt == CT - 1),
                                skip_group_check=True)
                        sblk = sim[bt][:, ts(j, mb)]
                        if copy_flip[0] % 2 == 0:
                            nc.vector.tensor_copy(sblk, psim)
                        else:
                            nc.scalar.copy(sblk, psim)
                        copy_flip[0] += 1
                        c8a = cand[bt][:, j * KPB:j * KPB + 8]
                        c8b = cand[bt][:, j * KPB + 8:j * KPB + 16]
                        nc.vector.max(c8a, sblk)
                        scr = pK.tile([BTW, mb], F32, tag="scr", bufs=1)
                        nc.vector.match_replace(
                            scr, in_to_replace=c8a, in_values=sblk,
                            imm_value=NEG)
                        nc.vector.max(c8b, scr)

                for mbi in range(NMB):
                    if mbi >= KT_BUFS:
                        emit_mm1(mbi - KT_BUFS)
                    ktb = pK.tile([128, KTPB, C], F32, tag="ktb",
                                  bufs=KTB_BUFS)
                    nc.sync.dma_start(
                        out=ktb,
                        in_=keys[mbi * mb:(mbi + 1) * mb].rearrange(
                            "(kt p) c -> p kt c", p=128))
                    kTt = pK.tile([128, CT, mb], F32, tag="kT",
                                  bufs=KT_BUFS)
                    kT_tiles[mbi] = kTt
                    for kt in range(KTPB):
                        ktile = ktb[:, kt]
                        kts = pK.tile([128, C], F32, tag="kts", bufs=2)
                        ssk = pK.tile([128, 1], F32, tag="ssk", bufs=2)
                        # kts doubles as the junk squares output here;
                        # the row-scale below overwrites it.
                        nc.scalar.activation(kts, ktile, AF.Square,
                                             accum_out=ssk)
                        kn = pK.tile([128, 1], F32, tag="kn", bufs=2)
                        nc.scalar.sqrt(kn, ssk)
                        rk = pK.tile([128, 1], F32, tag="rk", bufs=2)
                        nc.vector.reciprocal(rk, kn)
                        nc.vector.tensor_scalar_mul(kts, ktile, rk)
                        for dt in range(CT):
                            nc.tensor.matmul(
                                pkt[dt][:, ts(kt, 128)],
                                lhsT=kts[:, ts(dt, 128)], rhs=identity,
                                is_transpose=True,
                                start=True, stop=True, skip_group_check=True)
                    for dt in range(CT):
                        if dt % 2 == 0:
                            nc.vector.tensor_copy(kTt[:, dt], pkt[dt])
                        else:
                            nc.scalar.copy(kTt[:, dt], pkt[dt])
                    if mbi == NMB - 1:
                        # qag readback after all key DMAs so it never blocks
                        # the key stream (AG1b completes before keys drain).
                        for h in range(2):
                            for r in range(n_cores):
                                nc.sync.dma_start(
                                    out=qTt[:, :,
                                            r * BS + h * BS2:
                                            r * BS + (h + 1) * BS2],
                                    in_=qag_out[h][r].rearrange(
                                        "(ct p) b -> p ct b", p=128))
                for j in range(NMB - KT_BUFS, NMB):
                    emit_mm1(j)

            # ---------------- value stream (sync FIFO, behind keys) -------
            # Only the first VTB_BUFS value DMAs are issued ahead of the
            # candidate exchange: they fill fresh buffers and cannot stall
            # the FIFO.  The rest are emitted after the gc readback so their
            # buffer-reuse waits (on the W-phase fp16 casts) cannot block
            # cd_in/gc, which the W phase depends on.
            def emit_vtb(pV, vtbs, g):
                vtb = pV.tile([128, KTPB, C], F32, tag="vtb",
                              bufs=VTB_BUFS)
                nc.sync.dma_start(
                    out=vtb,
                    in_=vals[g * mb:(g + 1) * mb].rearrange(
                        "(kt p) c -> p kt c", p=128))
                vtbs.append(vtb)

            with tc.tile_pool(name="poolV", bufs=1) as pV:
                vtbs = []
                for g in range(VTB_BUFS):
                    emit_vtb(pV, vtbs, g)
                wexp = [pV.tile([BTW, MS], F32, name=f"wexp{i}")
                        for i in range(BT)]

                # ------------- Phase G: global top-K + softmax stats ------
                with (
                    tc.tile_pool(name="poolG", bufs=1) as pG,
                    tc.tile_pool(name="psumG", bufs=1, space="PSUM") as psG,
                ):
                    R = K // 8
                    # Q: rinv from the gathered queries
                    qsq = pG.tile([128, CT, B], F32, tag="qsq")
                    nc.scalar.square(qsq, qTt)
                    pss = psG.tile([1, B], F32, tag="pss")
                    for ct in range(CT):
                        nc.tensor.matmul(pss, lhsT=ones_col, rhs=qsq[:, ct],
                                         start=(ct == 0), stop=(ct == CT - 1))
                    nc.scalar.sqrt(qn_row, pss)
                    nc.vector.reciprocal(ri_row, qn_row)
                    for bt in range(BT):
                        psum_rt = psG.tile([BTW, 1], F32, tag="rt", bufs=2)
                        nc.tensor.matmul(
                            psum_rt, lhsT=ri_row[0:1, ts(bt, BTW)],
                            rhs=ones_col[0:1, 0:1], start=True, stop=True)
                        nc.vector.tensor_copy(rinv[bt], psum_rt)
                        loc = pG.tile([BTW, K], F32, tag="loc", bufs=2)
                        scr2 = pG.tile([BTW, NMB * KPB], F32, tag="scr2",
                                       bufs=2)
                        cur = cand[bt]
                        for r in range(R):
                            nc.vector.max(loc[:, r * 8:(r + 1) * 8], cur)
                            if r < R - 1:
                                nc.vector.match_replace(
                                    scr2,
                                    in_to_replace=loc[:, r * 8:(r + 1) * 8],
                                    in_values=cur, imm_value=NEG)
                                cur = scr2
                        nc.sync.dma_start(out=cd_in[ts(bt, BTW), :],
                                          in_=loc)
                        # local softmax bias: nb_l = -lmax * rinv
                        nc.vector.tensor_mul(nb_l[bt], loc[:, 0:1], rinv[bt])
                        nc.vector.tensor_scalar_mul(nb_l[bt], nb_l[bt], -1.0)
                    # exp with LOCAL stats — runs under the AG-cand latency;
                    # the global correction folds into rowfix (phase O).
                    for bt in range(BT):
                        nc.scalar.activation(wexp[bt], sim[bt], AF.Exp,
                                             bias=nb_l[bt], scale=rinv[bt])
                    nc.gpsimd.collective_compute(
                        "AllGather", ALU.bypass, replica_groups=RG,
                        ins=[cd_in.opt()], outs=[cd_out.opt()])
                    for bt in range(BT):
                        gc = pG.tile([BTW, n_cores * K], F32, tag="gc",
                                     bufs=2)
                        nc.sync.dma_start(
                            out=gc.rearrange("p (r k) -> p r k", r=n_cores),
                            in_=cd_out[:, ts(bt, BTW), :].rearrange(
                                "r p k -> p r k"))
                        scr3 = pG.tile([BTW, n_cores * K], F32, tag="scr3",
                                       bufs=2)
                        cur = gc
                        for r in range(R):
                            nc.vector.max(g32[bt][:, r * 8:(r + 1) * 8], cur)
                            if r < R - 1:
                                nc.vector.match_replace(
                                    scr3,
                                    in_to_replace=g32[bt][:,
                                                          r * 8:(r + 1) * 8],
                                    in_values=cur, imm_value=NEG)
                                cur = scr3
                        # stats: nbg = -gmax*rinv ; Z = sum exp((g-gmax)*rinv)
                        # rowfix = exp(nbg - nb_l - lnZ) applied in phase O
                        nbg = pG.tile([BTW, 1], F32, tag="nbg", bufs=2)
                        nc.vector.tensor_mul(nbg, g32[bt][:, 0:1], rinv[bt])
                        nc.vector.tensor_scalar_mul(nbg, nbg, -1.0)
                        ex = pG.tile([BTW, K], F32, tag="ex", bufs=2)
                        zz = pG.tile([BTW, 1], F32, tag="zz", bufs=2)
                        nc.scalar.activation(ex, g32[bt][:, 0:K], AF.Exp,
                                             bias=nbg, scale=rinv[bt],
                                             accum_out=zz)
                        lnz = pG.tile([BTW, 1], F32, tag="lnz", bufs=2)
                        nc.scalar.activation(lnz, zz, AF.Ln)
                        nc.vector.tensor_sub(bias2[bt], nbg, nb_l[bt])
                        nc.vector.tensor_sub(bias2[bt], bias2[bt], lnz)
                        nc.scalar.activation(rowfix[bt], bias2[bt], AF.Exp)

                # rest of the value stream (reuse-gated; see emit_vtb note)
                for g in range(VTB_BUFS, MT // KTPB):
                    emit_vtb(pV, vtbs, g)

                # ------------- Phase W: dense weights + matmul2 (fp16) ----
                with (
                    tc.tile_pool(name="poolW", bufs=1) as pW,
                    tc.tile_pool(name="psumW", bufs=1, space="PSUM") as psW,
                ):
                    for bt in range(BT):
                        nc.vector.scalar_tensor_tensor(
                            out=wexp[bt], in0=sim[bt],
                            scalar=g32[bt][:, K - 1:K], in1=wexp[bt],
                            op0=ALU.is_ge, op1=ALU.mult)
                    pm = [psW.tile([128, B], F32, tag=f"pm{dt}",
                                   name=f"pm{dt}") for dt in range(CT)]
                    vt16 = None
                    for mt in range(MT):
                        g, kt = mt // KTPB, mt % KTPB
                        if kt == 0:
                            vt16 = pW.tile([128, KTPB, C], F16, tag="vt16",
                                           bufs=2)
                            nc.vector.tensor_copy(vt16, vtbs[g])
                        pwt = psW.tile([128, B], F32, tag="pwt", bufs=3)
                        for bt in range(BT):
                            nc.tensor.matmul(
                                pwt[:, ts(bt, BTW)],
                                lhsT=wexp[bt][:, ts(mt, 128)],
                                rhs=identity, is_transpose=True,
                                start=True, stop=True, skip_group_check=True)
                        wT16 = pW.tile([128, B], F16, tag="wT16", bufs=3)
                        if mt % 2 == 0:
                            nc.vector.tensor_copy(wT16, pwt)
                        else:
                            nc.scalar.copy(wT16, pwt)
                        for dt in range(CT):
                            nc.tensor.matmul(
                                pm[dt], lhsT=vt16[:, kt, ts(dt, 128)],
                                rhs=wT16,
                                start=(mt == 0), stop=(mt == MT - 1),
                                skip_group_check=True)
                    for dt in range(CT):
                        nc.any.tensor_copy(mT[:, dt], pm[dt])

            # ---------------- Phase O: reduce-scatter + broadcast out -----
            with (
                tc.tile_pool(name="poolO", bufs=1) as pO,
                tc.tile_pool(name="psumO", bufs=1, space="PSUM") as psO,
            ):
                for bt in range(BT):
                    pmb = psO.tile([BTW, C], F32, tag="pmb", bufs=2)
                    for dt in range(CT):
                        nc.tensor.matmul(
                            pmb[:, ts(dt, 128)],
                            lhsT=mT[:, dt, ts(bt, BTW)],
                            rhs=identity, is_transpose=True,
                            start=True, stop=True, skip_group_check=True)
                    mrow = pO.tile([BTW, C], F32, tag="mrow", bufs=2)
                    # fold the local->global softmax correction in here
                    nc.scalar.mul(mrow, pmb, rowfix[bt])
                    nc.sync.dma_start(out=mb_dram[ts(bt, BTW), :], in_=mrow)
                nc.gpsimd.collective_compute(
                    "ReduceScatter", ALU.add, replica_groups=RG,
                    ins=[mb_dram.opt()], outs=[rs_out.opt()])
                mmy = pO.tile([BS, C], F32, tag="mmy", bufs=1)
                nc.sync.dma_start(out=mmy, in_=rs_out)
                for dt in range(CT):
                    pmt = psO.tile([128, BS], F32, tag="pmt", bufs=2)
                    nc.tensor.matmul(
                        pmt, lhsT=mmy[:, ts(dt, 128)],
                        rhs=identity[0:BS, 0:BS], is_transpose=True,
                        start=True, stop=True, skip_group_check=True)
                    nc.any.tensor_copy(mTmy[dt], pmt)
                for oi in range(BS // OPD):
                    ot = pO.tile([128, OPD, CT, HW], F32, tag="ot", bufs=2)
                    for bs_ in range(OPD):
                        b = oi * OPD + bs_
                        for dt in range(CT):
                            col = mTmy[dt][:, b:b + 1]
                            if dt < CT // 2:
                                nc.vector.tensor_scalar_mul(
                                    ot[:, bs_, dt], ones_hw, col)
                            else:
                                nc.scalar.mul(ot[:, bs_, dt], ones_hw, col)
                    nc.sync.dma_start(
                        out=out[oi * OPD:(oi + 1) * OPD].rearrange(
                            "b (ct p) hw -> p b ct hw", p=128),
                        in_=ot)

    nc.compile()
    return nc


_CACHE = {}
TRACE = False
LAST_RESULT = None


def _get(shape_key):
    if shape_key not in _CACHE:
        _CACHE[shape_key] = build(*shape_key)
    return _CACHE[shape_key]


def kernel(x, keys, values, topk, **_ignored):
    K = int(np.asarray(topk))
    B, C, H, W = x.shape
    M, D = keys.shape
    HW = H * W
    nc = _get((B, C, HW, M, K, N_CORES))
    BS, MS = B // N_CORES, M // N_CORES
    x3 = np.ascontiguousarray(x.reshape(B, C, HW)).astype(np.float32, copy=False)
    keys = np.ascontiguousarray(keys).astype(np.float32, copy=False)
    values = np.ascontiguousarray(values).astype(np.float32, copy=False)
    in_maps = [{
        "xs": x3[c * BS:(c + 1) * BS],
        "keys": keys[c * MS:(c + 1) * MS],
        "vals": values[c * MS:(c + 1) * MS],
    } for c in range(N_CORES)]
    global LAST_RESULT
    res = run_bass_kernel_spmd(nc, in_maps, core_ids=list(range(N_CORES)),
                               trace=TRACE)
    LAST_RESULT = res
    outs = [res.results[c]["out"] for c in range(N_CORES)]
    return np.concatenate(outs, axis=0).reshape(B, C, H, W)
